# revision 12
# baseline (speedup 1.0000x reference)
"""Trainium2 Bass kernel for nn_Attention_45835890982922.

Dense multi-head attention block:
    qkv = x @ w_qkv ; q,k layernormed per head (eps=1e-5), q scaled by D^-0.5
    out = softmax(q k^T) v ; y = concat_heads(out) @ w_proj + b_proj

Sharding over 8 NeuronCores: hybrid batch x tensor-parallel.
Core c handles batch b = c//2 and heads [6*(c%2), 6*(c%2)+6).
Each core computes a partial y^T (its 6 heads through the matching
w_proj rows); the host sums the two partials per batch and adds b_proj.

On-chip layout is feature-major (transposed): x^T, q^T, k^T [D, tokens],
so every matmul contraction lives on the partition axis with no per-head
transposes.  Softmax runs without max-subtraction (|S| <= ~8 after LN),
with the normalization sum obtained from an extra all-ones column
appended to v; the division is folded into the PSUM->SBUF epilogue of
the attention-output matmul.

dtypes: float32r (TensorE reduced fp32, ~1.5e-4) for qkv/S/stats/proj
matmuls, bf16 for exp(S) probabilities and v, fp32 accumulation in PSUM.
"""

from contextlib import ExitStack

import numpy as np

import concourse.bacc as bacc
import concourse.tile as tile
import concourse.mybir as mybir
from concourse.bass_utils import run_bass_kernel_spmd

F32 = mybir.dt.float32
F32R = mybir.dt.float32r
BF16 = mybir.dt.bfloat16
OP = mybir.AluOpType
AF = mybir.ActivationFunctionType

B, N, C, H, D = 4, 2048, 768, 12, 64
HL = H // 2              # 6 heads per core
CL = HL * D              # 384 local feature rows
P = 128
NKT = N // P             # 16 key tiles
QC = 1024                # query chunk for attention
NQC = N // QC
CT = C // P              # 6 contraction tiles over C
FT_QK = 2 * CL // P      # 6 feature tiles for q|k
KT3 = CL // P            # 3 contraction tiles over CL
LN_EPS = 1e-5
SCALE = float(D) ** -0.5


def _build(trivial_beta: bool):
    nc = bacc.Bacc("TRN2", target_bir_lowering=False, debug=False, num_devices=8)

    x_d = nc.dram_tensor("x", [N, C], F32, kind="ExternalInput").ap()
    wqk_d = nc.dram_tensor("wqk", [C, 2 * CL], F32R, kind="ExternalInput").ap()
    wv_d = nc.dram_tensor("wv", [C, CL], F32R, kind="ExternalInput").ap()
    wp_d = nc.dram_tensor("wp", [CL, C], F32R, kind="ExternalInput").ap()
    ident_d = nc.dram_tensor("ident", [P, P], F32, kind="ExternalInput").ap()
    bd6_d = nc.dram_tensor("bd6", [CL, HL], F32R, kind="ExternalInput").ap()
    bc6_d = nc.dram_tensor("bc6", [HL, CL], F32R, kind="ExternalInput").ap()
    gb_d = nc.dram_tensor("gb", [CL, 4], F32, kind="ExternalInput").ap()
    y_d = nc.dram_tensor("y", [C, N], F32, kind="ExternalOutput").ap()

    with tile.TileContext(nc) as tc, ExitStack() as top:
        top.enter_context(
            nc.allow_low_precision(reason="f32r/bf16 staging is intentional")
        )
        const = top.enter_context(tc.tile_pool(name="const", bufs=1))
        vp = top.enter_context(tc.tile_pool(name="vpool", bufs=1))

        ident = const.tile([P, P], F32)
        nc.sync.dma_start(ident[:], ident_d)
        bd6 = const.tile([P, KT3, HL], F32R)
        nc.sync.dma_start(bd6[:], bd6_d.rearrange("(t p) h -> p t h", p=P))
        bc6 = const.tile([HL, CL], F32R)
        nc.sync.dma_start(bc6[:], bc6_d)
        gb = const.tile([P, KT3, 4], F32)
        nc.sync.dma_start(gb[:], gb_d.rearrange("(t p) c -> p t c", p=P))

        # v token-major bf16 with per-head all-ones column: [p, ttile, h*65+e]
        v_sb = vp.tile([P, NKT, HL * 65], BF16)
        v_view = v_sb[:].rearrange("p t (h e) -> p t h e", h=HL)
        nc.gpsimd.memset(v_view[:, :, :, 64:65], 1.0)

        # q^T | k^T feature-major accumulator: [p, ft, tokens]; ft 0-2 q, 3-5 k.
        # LayerNorm is applied in-place, so this same tile later holds qhat/khat.
        qkp = top.enter_context(tc.tile_pool(name="qkraw", bufs=1))
        qk_raw = qkp.tile([P, FT_QK, N], F32R)
        # LN smalls live through phases B-C only
        sAC = top.enter_context(ExitStack())
        smp = sAC.enter_context(tc.tile_pool(name="smalls", bufs=1))

        # ================ phase A: x^T, qkv, v ================
        with ExitStack() as sA:
            pA = sA.enter_context(tc.tile_pool(name="phA", bufs=1))
            pAx = sA.enter_context(tc.tile_pool(name="phAx", bufs=2))
            psA = sA.enter_context(tc.tile_pool(name="psA", bufs=2, space="PSUM"))

            wqk_r = pA.tile([P, CT, 2 * CL], F32R)
            nc.sync.dma_start(wqk_r[:], wqk_d.rearrange("(t p) f -> p t f", p=P))
            wv_r = pA.tile([P, CT, CL], F32R)
            nc.sync.dma_start(wv_r[:], wv_d.rearrange("(t p) f -> p t f", p=P))

            x_t = pA.tile([P, CT, N], F32R)           # [c%128, ctile, token]
            for ct in range(CT):
                xs = pAx.tile([P, NKT, P], F32, tag="xslice")
                nc.sync.dma_start(
                    xs[:], x_d.rearrange("(t p) c -> p t c", p=P)[:, :, ct * P:(ct + 1) * P]
                )
                for tt in range(NKT):
                    pst = psA.tile([P, P], F32, tag="ps_tr")
                    nc.tensor.transpose(pst[:], xs[:, tt, :], ident[:])
                    nc.vector.tensor_copy(x_t[:, ct, tt * P:(tt + 1) * P], pst[:])

            for ft in range(FT_QK):
                for nk in range(N // 512):
                    ps = psA.tile([P, 512], F32, tag="ps_qkv")
                    for kt in range(CT):
                        nc.tensor.matmul(
                            ps[:],
                            wqk_r[:, kt, ft * P:(ft + 1) * P],
                            x_t[:, kt, nk * 512:(nk + 1) * 512],
                            start=(kt == 0),
                            stop=(kt == CT - 1),
                        )
                    nc.vector.tensor_copy(qk_raw[:, ft, nk * 512:(nk + 1) * 512], ps[:])

            for tt in range(NKT):
                psv = psA.tile([P, CL], F32, tag="ps_v")
                for kt in range(CT):
                    nc.tensor.matmul(
                        psv[:],
                        x_t[:, kt, tt * P:(tt + 1) * P],
                        wv_r[:, kt, :],
                        start=(kt == 0),
                        stop=(kt == CT - 1),
                    )
                nc.vector.tensor_copy(
                    v_view[:, tt, :, 0:64],
                    psv[:].rearrange("p (h d) -> p h d", h=HL),
                )

        # ================ phase B: LN stats ================
        # per-token sums over D via block-diagonal ones matmuls -> [6, tokens]
        with ExitStack() as sB:
            pB = sB.enter_context(tc.tile_pool(name="phB", bufs=2))
            psB = sB.enter_context(tc.tile_pool(name="psB", bufs=2, space="PSUM"))

            sm_mu = [smp.tile([HL, N], F32R, tag=f"mu{s}", name=f"sm_mu{s}") for s in range(2)]
            sm_rst = [smp.tile([HL, N], F32R, tag=f"rst{s}", name=f"sm_rst{s}") for s in range(2)]

            for s in range(2):
                for nk in range(N // 512):
                    psm = psB.tile([HL, 512], F32, tag="ps_stat")
                    for kt in range(KT3):
                        nc.tensor.matmul(
                            psm[:],
                            bd6[:, kt, :],
                            qk_raw[:, 3 * s + kt, nk * 512:(nk + 1) * 512],
                            start=(kt == 0),
                            stop=(kt == KT3 - 1),
                        )
                    nc.vector.tensor_scalar_mul(
                        sm_mu[s][:, nk * 512:(nk + 1) * 512], psm[:], 1.0 / D
                    )
            for s in range(2):
                for nk in range(N // 512):
                    psm = psB.tile([HL, 512], F32, tag="ps_stat")
                    for kt in range(KT3):
                        sq = pB.tile([P, 512], F32R, tag="sq")
                        nc.scalar.square(
                            sq[:], qk_raw[:, 3 * s + kt, nk * 512:(nk + 1) * 512]
                        )
                        nc.tensor.matmul(
                            psm[:],
                            bd6[:, kt, :],
                            sq[:],
                            start=(kt == 0),
                            stop=(kt == KT3 - 1),
                        )
                    nc.vector.tensor_scalar_mul(
                        sm_rst[s][:, nk * 512:(nk + 1) * 512], psm[:], 1.0 / D
                    )
            # var = E[x^2] - mu^2 ; rstd = 1/sqrt(var+eps); fold D^-0.5 into q
            for s in range(2):
                tmp = smp.tile([HL, N], F32, tag=f"tmp{s}")
                nc.vector.tensor_tensor(tmp[:], sm_mu[s][:], sm_mu[s][:], OP.mult)
                # var + eps = (E[x^2] + eps) - mu^2
                nc.vector.scalar_tensor_tensor(
                    tmp[:], sm_rst[s][:], LN_EPS, tmp[:],
                    op0=OP.add, op1=OP.subtract,
                )
                nc.scalar.activation(tmp[:], tmp[:], AF.Sqrt)
                nc.vector.reciprocal(sm_rst[s][:], tmp[:])
            nc.vector.tensor_scalar_mul(sm_rst[0][:], sm_rst[0][:], SCALE)

        # ================ phase C: LN apply (in-place into qk_raw) ================
        # qhat = ((raw - mu_bcast) * gamma) * rstd_bcast [+ beta]
        hat = qk_raw
        with ExitStack() as sC:
            pC = sC.enter_context(tc.tile_pool(name="phC", bufs=2))
            psC = sC.enter_context(tc.tile_pool(name="psC", bufs=2, space="PSUM"))
            for ft in range(FT_QK):
                s = ft // 3
                blk = ft % 3
                for nk in range(N // 512):
                    sl = slice(nk * 512, (nk + 1) * 512)
                    bmu = psC.tile([P, 512], F32, tag="bmu")
                    nc.tensor.matmul(
                        bmu[:], bc6[:, blk * P:(blk + 1) * P], sm_mu[s][:, sl],
                        start=True, stop=True,
                    )
                    brs = psC.tile([P, 512], F32, tag="brs")
                    nc.tensor.matmul(
                        brs[:], bc6[:, blk * P:(blk + 1) * P], sm_rst[s][:, sl],
                        start=True, stop=True,
                    )
                    tdiff = pC.tile([P, 512], F32, tag="tdiff")
                    nc.vector.tensor_tensor(
                        tdiff[:], qk_raw[:, ft, sl], bmu[:], OP.subtract
                    )
                    nc.vector.scalar_tensor_tensor(
                        hat[:, ft, sl],
                        tdiff[:],
                        gb[:, blk, 2 * s:2 * s + 1],
                        brs[:],
                        op0=OP.mult,
                        op1=OP.mult,
                    )
                    if not trivial_beta:
                        nc.vector.tensor_scalar_add(
                            hat[:, ft, sl], hat[:, ft, sl],
                            gb[:, blk, 2 * s + 1:2 * s + 2],
                        )

        sAC.close()  # free LN smalls before attention

        # ================ phase D: attention ================
        outp = top.enter_context(tc.tile_pool(name="outT", bufs=1))
        out_t = outp.tile([P, KT3, N], F32R)          # out^T feature-major
        with ExitStack() as sD:
            expp = sD.enter_context(tc.tile_pool(name="expp", bufs=2))
            pD = sD.enter_context(tc.tile_pool(name="phD", bufs=2))
            psS = sD.enter_context(tc.tile_pool(name="psS", bufs=2, space="PSUM"))
            psO = sD.enter_context(tc.tile_pool(name="psO", bufs=2, space="PSUM"))
            for h in range(HL):
                ht = h // 2
                hr = 64 * (h % 2)
                for qc in range(NQC):
                    exp_t = expp.tile([P, NKT, QC], BF16, tag="exp")
                    for kt in range(NKT):
                        ps_st = psS.tile([P, QC], F32, tag="ps_s")
                        for nk in range(QC // 512):
                            nc.tensor.matmul(
                                ps_st[:, nk * 512:(nk + 1) * 512],
                                hat[hr:hr + 64, 3 + ht, kt * P:(kt + 1) * P],
                                hat[hr:hr + 64, ht,
                                    qc * QC + nk * 512:qc * QC + (nk + 1) * 512],
                                start=True,
                                stop=True,
                            )
                        nc.scalar.activation(exp_t[:, kt, :], ps_st[:], AF.Exp)
                    ps_o = psO.tile([65, QC], F32, tag="ps_o")
                    for kt in range(NKT):
                        for nk in range(QC // 512):
                            nc.tensor.matmul(
                                ps_o[:, nk * 512:(nk + 1) * 512],
                                v_view[:, kt, h, :],
                                exp_t[:, kt, nk * 512:(nk + 1) * 512],
                                start=(kt == 0),
                                stop=(kt == NKT - 1),
                            )
                    rc = pD.tile([1, QC], F32, tag="recip")
                    nc.vector.reciprocal(rc[:], ps_o[64:65, :])
                    rcb = pD.tile([64, QC], F32, tag="recipb")
                    nc.gpsimd.partition_broadcast(rcb[:], rc[:])
                    nc.vector.tensor_tensor(
                        out_t[hr:hr + 64, ht, qc * QC:(qc + 1) * QC],
                        ps_o[0:64, :],
                        rcb[:],
                        OP.mult,
                    )

        # ================ phase E: output projection ================
        with ExitStack() as sE:
            pE = sE.enter_context(tc.tile_pool(name="phE", bufs=2))
            wpp = sE.enter_context(tc.tile_pool(name="wpp", bufs=1))
            psE = sE.enter_context(tc.tile_pool(name="psE", bufs=2, space="PSUM"))
            wp_r = wpp.tile([P, KT3, C], F32R)
            nc.sync.dma_start(wp_r[:], wp_d.rearrange("(t p) f -> p t f", p=P))
            for mt in range(C // P):
                y_sb = pE.tile([P, N], F32, tag="y")
                for nk in range(N // 512):
                    ps_y = psE.tile([P, 512], F32, tag="ps_y")
                    for kt in range(KT3):
                        nc.tensor.matmul(
                            ps_y[:],
                            wp_r[:, kt, mt * P:(mt + 1) * P],
                            out_t[:, kt, nk * 512:(nk + 1) * 512],
                            start=(kt == 0),
                            stop=(kt == KT3 - 1),
                        )
                    nc.vector.tensor_copy(y_sb[:, nk * 512:(nk + 1) * 512], ps_y[:])
                nc.sync.dma_start(y_d[mt * P:(mt + 1) * P, :], y_sb[:])

    nc.compile()
    return nc


def _host_prep(x, w_qkv, q_gamma, q_beta, k_gamma, k_beta, w_proj):
    """Per-core input maps."""
    ident = np.eye(P, dtype=np.float32)
    bd6 = np.zeros((CL, HL), dtype=np.float32)
    for h in range(HL):
        bd6[h * D:(h + 1) * D, h] = 1.0
    bc6 = np.ascontiguousarray(bd6.T)
    in_maps = []
    for c in range(8):
        b = c // 2
        half = c % 2
        heads = range(HL * half, HL * half + HL)
        wq = np.concatenate([w_qkv[:, h * D:(h + 1) * D] for h in heads], axis=1)
        wk = np.concatenate(
            [w_qkv[:, C + h * D:C + (h + 1) * D] for h in heads], axis=1
        )
        wv = np.concatenate(
            [w_qkv[:, 2 * C + h * D:2 * C + (h + 1) * D] for h in heads], axis=1
        )
        wqk = np.ascontiguousarray(np.concatenate([wq, wk], axis=1))
        wp = np.ascontiguousarray(w_proj[CL * half:CL * half + CL, :])
        gb = np.stack(
            [
                np.tile(q_gamma, HL),
                np.tile(q_beta, HL) * SCALE,
                np.tile(k_gamma, HL),
                np.tile(k_beta, HL),
            ],
            axis=1,
        ).astype(np.float32)
        in_maps.append(
            {
                "x": np.ascontiguousarray(x[b]),
                "wqk": wqk,
                "wv": np.ascontiguousarray(wv),
                "wp": wp,
                "ident": ident,
                "bd6": bd6,
                "bc6": bc6,
                "gb": gb,
            }
        )
    return in_maps


def kernel(x, w_qkv, q_gamma, q_beta, k_gamma, k_beta, w_proj, b_proj):
    x = np.asarray(x, dtype=np.float32)
    w_qkv = np.asarray(w_qkv, dtype=np.float32)
    q_gamma = np.asarray(q_gamma, dtype=np.float32)
    q_beta = np.asarray(q_beta, dtype=np.float32)
    k_gamma = np.asarray(k_gamma, dtype=np.float32)
    k_beta = np.asarray(k_beta, dtype=np.float32)
    w_proj = np.asarray(w_proj, dtype=np.float32)
    b_proj = np.asarray(b_proj, dtype=np.float32)

    trivial_beta = bool(np.all(q_beta == 0.0) and np.all(k_beta == 0.0))
    nc = _build(trivial_beta)
    in_maps = _host_prep(x, w_qkv, q_gamma, q_beta, k_gamma, k_beta, w_proj)
    res = run_bass_kernel_spmd(nc, in_maps, core_ids=list(range(8)))

    y = np.empty((B, N, C), dtype=np.float32)
    for b in range(B):
        yt = res.results[2 * b]["y"] + res.results[2 * b + 1]["y"]
        y[b] = yt.T + b_proj[None, :]
    return y


if __name__ == "__main__":
    rng = np.random.default_rng(0)
    out = kernel(
        rng.standard_normal((B, N, C), dtype=np.float32),
        (rng.standard_normal((C, 3 * C)) * C ** -0.5).astype(np.float32),
        np.ones(D, np.float32),
        np.zeros(D, np.float32),
        np.ones(D, np.float32),
        np.zeros(D, np.float32),
        (rng.standard_normal((C, C)) * C ** -0.5).astype(np.float32),
        np.zeros(C, np.float32),
    )
    print("ok", out.shape, float(np.abs(out).mean()))


# revision 26
# speedup vs baseline: 13.0350x; 13.0350x over previous
"""Trainium2 Bass kernel for nn_Attention_45835890982922.

Dense multi-head attention block:
    qkv = x @ w_qkv ; q,k layernormed per head (eps=1e-5), q scaled by D^-0.5
    out = softmax(q k^T) v ; y = concat_heads(out) @ w_proj + b_proj

Sharding over 8 NeuronCores: hybrid batch x tensor-parallel.
Core c handles batch b = c//2 and heads [6*(c%2), 6*(c%2)+6).
Each core computes a partial y^T (its 6 heads through the matching
w_proj rows); the host sums the two partials per batch and adds b_proj.

On-chip layout is feature-major (transposed): x^T, q^T, k^T [D, tokens],
so every matmul contraction lives on the partition axis with no per-head
transposes.  Softmax runs without max-subtraction (|S| <= ~8 after LN),
with the normalization sum obtained from an extra all-ones column
appended to v; the division is folded into the PSUM->SBUF epilogue of
the attention-output matmul.

dtypes: float32r (TensorE reduced fp32, ~1.5e-4) for qkv/S/stats/proj
matmuls, bf16 for exp(S) probabilities and v, fp32 accumulation in PSUM.
"""

from contextlib import ExitStack

import numpy as np

import concourse.bacc as bacc
import concourse.tile as tile
import concourse.mybir as mybir
from concourse.bass_utils import run_bass_kernel_spmd

F32 = mybir.dt.float32
F32R = mybir.dt.float32r
BF16 = mybir.dt.bfloat16
OP = mybir.AluOpType
AF = mybir.ActivationFunctionType

B, N, C, H, D = 4, 2048, 768, 12, 64
HL = H // 2              # 6 heads per core
CL = HL * D              # 384 local feature rows
P = 128
NKT = N // P             # 16 key tiles
QC = 1024                # query chunk for attention
NQC = N // QC
CT = C // P              # 6 contraction tiles over C
FT_QK = 2 * CL // P      # 6 feature tiles for q|k
KT3 = CL // P            # 3 contraction tiles over CL
LN_EPS = 1e-5
SCALE = float(D) ** -0.5

# ablation knobs (timing experiments only; wrong output when not default)
ABL_HEADS = HL
ABL_QKV = True
ABL_ATTN = True
ABL_PROJ = True
ABL_SHIFT = False
ABL_EXPFN = True
ABL_LN = True
PSS_BUFS = 2
PSO_BUFS = 2
ABL_EPI = True


def _build(trivial_beta: bool, repeat: int = 1):
    nc = bacc.Bacc("TRN2", target_bir_lowering=False, debug=False, num_devices=8)

    x_d = nc.dram_tensor("x", [N, C], F32, kind="ExternalInput").ap()
    wqk_d = nc.dram_tensor("wqk", [C, 2 * CL], F32R, kind="ExternalInput").ap()
    wv_d = nc.dram_tensor("wv", [C, CL], F32R, kind="ExternalInput").ap()
    wp_d = nc.dram_tensor("wp", [CL, C], F32R, kind="ExternalInput").ap()
    ident_d = nc.dram_tensor("ident", [P, P], F32, kind="ExternalInput").ap()
    bd6_d = nc.dram_tensor("bd6", [CL, HL], F32R, kind="ExternalInput").ap()
    bc6_d = nc.dram_tensor("bc6", [HL, CL], F32R, kind="ExternalInput").ap()
    gb_d = nc.dram_tensor("gb", [CL, 4], F32, kind="ExternalInput").ap()
    sh_d = nc.dram_tensor("sh64", [P, P], F32R, kind="ExternalInput").ap()
    y_d = nc.dram_tensor("y", [C, N], F32, kind="ExternalOutput").ap()

    with tile.TileContext(nc) as tc, ExitStack() as top:
        top.enter_context(
            nc.allow_low_precision(reason="f32r/bf16 staging is intentional")
        )
        const = top.enter_context(tc.tile_pool(name="const", bufs=1))

        ident = const.tile([P, P], F32)
        nc.sync.dma_start(ident[:], ident_d)
        bd6 = const.tile([P, KT3, HL], F32R)
        nc.sync.dma_start(bd6[:], bd6_d.rearrange("(t p) h -> p t h", p=P))
        bc6 = const.tile([HL, CL], F32R)
        nc.sync.dma_start(bc6[:], bc6_d)
        gb = const.tile([P, KT3, 4], F32)
        nc.sync.dma_start(gb[:], gb_d.rearrange("(t p) c -> p t c", p=P))
        sh64 = const.tile([P, P], F32R)
        nc.sync.dma_start(sh64[:], sh_d)

        for rep in range(repeat):
            _emit_iteration(
                nc, tc, rep, trivial_beta,
                x_d, wqk_d, wv_d, wp_d, y_d, ident, bd6, bc6, gb, sh64,
            )

    nc.compile()
    return nc


def _emit_iteration(nc, tc, rep, trivial_beta,
                    x_d, wqk_d, wv_d, wp_d, y_d, ident, bd6, bc6, gb, sh64):
    with ExitStack() as top:
        vp = top.enter_context(tc.tile_pool(name=f"vpool{rep}", bufs=1))
        # v token-major bf16 with per-head all-ones column: [p, ttile, h*65+e]
        v_sb = vp.tile([P, NKT, HL * 65], BF16)
        v_view = v_sb[:].rearrange("p t (h e) -> p t h e", h=HL)
        nc.gpsimd.memset(v_view[:, :, :, 64:65], 1.0)

        # q^T | k^T feature-major accumulator: [p, ft, tokens]; ft 0-2 q, 3-5 k.
        # LayerNorm is applied in-place, so this same tile later holds qhat/khat.
        qkp = top.enter_context(tc.tile_pool(name=f"qkraw{rep}", bufs=1))
        qk_fts = [
            qkp.tile([P, N], F32R, name=f"qk_ft{ft}_{rep}") for ft in range(FT_QK)
        ]

        class _FtView:
            """hat[p_slice, ft, col_slice] -> per-ft tile AP."""
            def __init__(self, tiles):
                self.tiles = tiles
            def __getitem__(self, idx):
                p, ft, col = idx
                return self.tiles[ft][p, col]

        qk_raw = _FtView(qk_fts)
        # LN smalls live through phases B-C only
        sAC = top.enter_context(ExitStack())
        smp = sAC.enter_context(tc.tile_pool(name=f"smalls{rep}", bufs=1))

        # ================ phase A: x^T, qkv, v ================
        with ExitStack() as sA:
            pA = sA.enter_context(tc.tile_pool(name=f"phA{rep}", bufs=1))
            pAx = sA.enter_context(tc.tile_pool(name=f"phAx{rep}", bufs=2))
            psA = sA.enter_context(tc.tile_pool(name=f"psA{rep}", bufs=2, space="PSUM"))

            wqk_r = pA.tile([P, CT, 2 * CL], F32R)
            nc.sync.dma_start(wqk_r[:], wqk_d.rearrange("(t p) f -> p t f", p=P))
            wv_r = pA.tile([P, CT, CL], F32R)
            nc.sync.dma_start(wv_r[:], wv_d.rearrange("(t p) f -> p t f", p=P))

            x_t = pA.tile([P, CT, N], F32R)           # [c%128, ctile, token]
            for ct in range(CT):
                xs = pAx.tile([P, NKT, P], F32, tag="xslice")
                nc.sync.dma_start(
                    xs[:], x_d.rearrange("(t p) c -> p t c", p=P)[:, :, ct * P:(ct + 1) * P]
                )
                for tt in range(NKT):
                    pst = psA.tile([P, P], F32, tag="ps_tr")
                    nc.tensor.transpose(pst[:], xs[:, tt, :], ident[:])
                    nc.vector.tensor_copy(x_t[:, ct, tt * P:(tt + 1) * P], pst[:])

            for ft in range(FT_QK if ABL_QKV else 0):
                for nk in range(N // 512):
                    ps = psA.tile([P, 512], F32, tag="ps_qkv")
                    for kt in range(CT):
                        nc.tensor.matmul(
                            ps[:],
                            wqk_r[:, kt, ft * P:(ft + 1) * P],
                            x_t[:, kt, nk * 512:(nk + 1) * 512],
                            start=(kt == 0),
                            stop=(kt == CT - 1),
                        )
                    nc.vector.tensor_copy(qk_raw[:, ft, nk * 512:(nk + 1) * 512], ps[:])

            for tt in range(NKT):
                psv = psA.tile([P, CL], F32, tag="ps_v")
                for kt in range(CT):
                    nc.tensor.matmul(
                        psv[:],
                        x_t[:, kt, tt * P:(tt + 1) * P],
                        wv_r[:, kt, :],
                        start=(kt == 0),
                        stop=(kt == CT - 1),
                    )
                nc.vector.tensor_copy(
                    v_view[:, tt, :, 0:64],
                    psv[:].rearrange("p (h d) -> p h d", h=HL),
                )

        # ================ phase B: LN stats ================
        # per-token sums over D via block-diagonal ones matmuls -> [6, tokens]
        with ExitStack() as sB:
            pB = sB.enter_context(tc.tile_pool(name=f"phB{rep}", bufs=2))
            psB = sB.enter_context(tc.tile_pool(name=f"psB{rep}", bufs=2, space="PSUM"))

            sm_mu = [smp.tile([HL, N], F32R, tag=f"mu{s}", name=f"sm_mu{s}_{rep}") for s in range(2)]
            sm_rst = [smp.tile([HL, N], F32R, tag=f"rst{s}", name=f"sm_rst{s}_{rep}") for s in range(2)]

            for s in range(2 if ABL_LN else 0):
                for nk in range(N // 512):
                    psm = psB.tile([HL, 512], F32, tag="ps_stat")
                    for kt in range(KT3):
                        nc.tensor.matmul(
                            psm[:],
                            bd6[:, kt, :],
                            qk_raw[:, 3 * s + kt, nk * 512:(nk + 1) * 512],
                            start=(kt == 0),
                            stop=(kt == KT3 - 1),
                        )
                    nc.vector.tensor_scalar_mul(
                        sm_mu[s][:, nk * 512:(nk + 1) * 512], psm[:], 1.0 / D
                    )
            for s in range(2 if ABL_LN else 0):
                for nk in range(N // 512):
                    psm = psB.tile([HL, 512], F32, tag="ps_stat")
                    for kt in range(KT3):
                        sq = pB.tile([P, 512], F32R, tag="sq")
                        nc.scalar.square(
                            sq[:], qk_raw[:, 3 * s + kt, nk * 512:(nk + 1) * 512]
                        )
                        nc.tensor.matmul(
                            psm[:],
                            bd6[:, kt, :],
                            sq[:],
                            start=(kt == 0),
                            stop=(kt == KT3 - 1),
                        )
                    nc.vector.tensor_scalar_mul(
                        sm_rst[s][:, nk * 512:(nk + 1) * 512], psm[:], 1.0 / D
                    )
            # var = E[x^2] - mu^2 ; rstd = 1/sqrt(var+eps); fold D^-0.5 into q
            for s in range(2 if ABL_LN else 0):
                tmp = smp.tile([HL, N], F32, tag=f"tmp{s}", name=f"tmp{s}_{rep}")
                nc.vector.tensor_tensor(tmp[:], sm_mu[s][:], sm_mu[s][:], OP.mult)
                # var + eps = (E[x^2] + eps) - mu^2
                nc.vector.scalar_tensor_tensor(
                    tmp[:], sm_rst[s][:], LN_EPS, tmp[:],
                    op0=OP.add, op1=OP.subtract,
                )
                nc.scalar.activation(tmp[:], tmp[:], AF.Sqrt)
                nc.vector.reciprocal(sm_rst[s][:], tmp[:])
            if ABL_LN:
                nc.vector.tensor_scalar_mul(sm_rst[0][:], sm_rst[0][:], SCALE)

        # ================ phase C: LN apply (in-place into qk_raw) ============
        # qhat = ((raw - mu_bcast) * gamma) * rstd_bcast [+ beta]
        hat = qk_raw
        with ExitStack() as sC:
            pC = sC.enter_context(tc.tile_pool(name=f"phC{rep}", bufs=2))
            psC = sC.enter_context(tc.tile_pool(name=f"psC{rep}", bufs=2, space="PSUM"))
            for ft in range(FT_QK if ABL_LN else 0):
                s = ft // 3
                blk = ft % 3
                for nk in range(N // 512):
                    sl = slice(nk * 512, (nk + 1) * 512)
                    bmu = psC.tile([P, 512], F32, tag="bmu")
                    nc.tensor.matmul(
                        bmu[:], bc6[:, blk * P:(blk + 1) * P], sm_mu[s][:, sl],
                        start=True, stop=True,
                    )
                    brs = psC.tile([P, 512], F32, tag="brs")
                    nc.tensor.matmul(
                        brs[:], bc6[:, blk * P:(blk + 1) * P], sm_rst[s][:, sl],
                        start=True, stop=True,
                    )
                    tdiff = pC.tile([P, 512], F32, tag="tdiff")
                    nc.vector.tensor_tensor(
                        tdiff[:], qk_raw[:, ft, sl], bmu[:], OP.subtract
                    )
                    nc.vector.scalar_tensor_tensor(
                        hat[:, ft, sl],
                        tdiff[:],
                        gb[:, blk, 2 * s:2 * s + 1],
                        brs[:],
                        op0=OP.mult,
                        op1=OP.mult,
                    )
                    if not trivial_beta:
                        nc.vector.tensor_scalar_add(
                            hat[:, ft, sl], hat[:, ft, sl],
                            gb[:, blk, 2 * s + 1:2 * s + 2],
                        )

        sAC.close()  # free LN smalls before attention

        # ================ phase C2: rotated copy of qhat|khat ================
        # hat_sh[p] = hat[(p+64)%128]: kt-odd S matmuls read it so adjacent
        # S matmuls hit different PE row groups and overlap.
        shp = top.enter_context(tc.tile_pool(name=f"shp{rep}", bufs=1))
        hat_sh = shp.tile([P, FT_QK, N], F32R, name=f"hat_sh{rep}") if ABL_SHIFT else hat
        with ExitStack() as sC2:
            psC2 = sC2.enter_context(tc.tile_pool(name=f"psC2{rep}", bufs=2, space="PSUM"))
            for t in range(FT_QK if ABL_SHIFT else 0):
                for nk in range(N // 512):
                    pssh = psC2.tile([P, 512], F32, tag="ps_sh")
                    nc.tensor.matmul(
                        pssh[:], sh64[:],
                        hat[:, t, nk * 512:(nk + 1) * 512],
                        start=True, stop=True,
                    )
                    nc.vector.tensor_copy(
                        hat_sh[:, t, nk * 512:(nk + 1) * 512], pssh[:]
                    )

        # ================ phase D: attention ================
        outp = top.enter_context(tc.tile_pool(name=f"outT{rep}", bufs=1))
        out_fts = [
            outp.tile([P, N], F32R, name=f"out_ft{t}_{rep}") for t in range(KT3)
        ]
        out_t = _FtView(out_fts)                      # out^T feature-major
        with ExitStack() as sD:
            expp = sD.enter_context(tc.tile_pool(name=f"expp{rep}", bufs=4))
            pD = sD.enter_context(tc.tile_pool(name=f"phD{rep}", bufs=2))
            psS = sD.enter_context(tc.tile_pool(name=f"psS{rep}", bufs=PSS_BUFS, space="PSUM"))
            psO = sD.enter_context(tc.tile_pool(name=f"psO{rep}", bufs=PSO_BUFS, space="PSUM"))
            HK = NKT // 2

            def emit_s_exp(h, qc):
                ht = h // 2
                hr = 64 * (h % 2)
                exp_halves = []
                for half in range(2):
                    exp_h = expp.tile(
                        [P, HK, QC], BF16, tag="exp", name=f"exp_{rep}_{h}_{qc}_{half}"
                    )
                    exp_halves.append(exp_h)
                    for kt in range(half * HK, (half + 1) * HK):
                        ps_st = psS.tile([P, QC], F32, tag="ps_s")
                        if kt % 2 == 0 or not ABL_SHIFT:
                            lhs = hat[hr:hr + 64, 3 + ht, kt * P:(kt + 1) * P]
                            rhs = hat[hr:hr + 64, ht, qc * QC:(qc + 1) * QC]
                        else:
                            lhs = hat_sh[64 - hr:128 - hr, 3 + ht,
                                         kt * P:(kt + 1) * P]
                            rhs = hat_sh[64 - hr:128 - hr, ht,
                                         qc * QC:(qc + 1) * QC]
                        for nk in range(QC // 512):
                            nc.tensor.matmul(
                                ps_st[:, nk * 512:(nk + 1) * 512],
                                lhs,
                                rhs[:, nk * 512:(nk + 1) * 512],
                                start=True,
                                stop=True,
                            )
                        nc.scalar.activation(
                            exp_h[:, kt - half * HK, :], ps_st[:],
                            AF.Exp if ABL_EXPFN else AF.Copy,
                        )
                return exp_halves

            def emit_pv(h, qc, exp_halves):
                ht = h // 2
                hr = 64 * (h % 2)
                ps_o = psO.tile([65, QC], F32, tag="ps_o")
                for kt in range(NKT):
                    for nk in range(QC // 512):
                        nc.tensor.matmul(
                            ps_o[:, nk * 512:(nk + 1) * 512],
                            v_view[:, kt, h, :],
                            exp_halves[kt // HK][:, kt % HK,
                                                 nk * 512:(nk + 1) * 512],
                            start=(kt == 0),
                            stop=(kt == NKT - 1),
                        )
                if ABL_EPI:
                    rc = pD.tile([1, QC], F32, tag="recip")
                    nc.vector.reciprocal(rc[:], ps_o[64:65, :])
                    rcb = pD.tile([64, QC], F32, tag="recipb")
                    nc.gpsimd.partition_broadcast(rcb[:], rc[:])
                    nc.vector.tensor_tensor(
                        out_t[hr:hr + 64, ht, qc * QC:(qc + 1) * QC],
                        ps_o[0:64, :],
                        rcb[:],
                        OP.mult,
                    )
                else:
                    nc.vector.tensor_copy(
                        out_t[hr:hr + 64, ht, qc * QC:(qc + 1) * QC],
                        ps_o[0:64, :],
                    )

            # software pipeline: next chunk's S/exp is emitted before this
            # chunk's PV so the PE feeds ACT continuously.
            pending = None
            for h in range(ABL_HEADS if ABL_ATTN else 0):
                for qc in range(NQC):
                    eh = emit_s_exp(h, qc)
                    if pending is not None:
                        emit_pv(*pending)
                    pending = (h, qc, eh)
            if pending is not None:
                emit_pv(*pending)

        # ================ phase E: output projection ================
        with ExitStack() as sE:
            pE = sE.enter_context(tc.tile_pool(name=f"phE{rep}", bufs=2))
            wpp = sE.enter_context(tc.tile_pool(name=f"wpp{rep}", bufs=1))
            psE = sE.enter_context(tc.tile_pool(name=f"psE{rep}", bufs=2, space="PSUM"))
            wp_r = wpp.tile([P, KT3, C], F32R)
            nc.sync.dma_start(wp_r[:], wp_d.rearrange("(t p) f -> p t f", p=P))
            for mt in range(C // P if ABL_PROJ else 0):
                y_sb = pE.tile([P, N], F32, tag="y")
                for nk in range(N // 512):
                    ps_y = psE.tile([P, 512], F32, tag="ps_y")
                    for kt in range(KT3):
                        nc.tensor.matmul(
                            ps_y[:],
                            wp_r[:, kt, mt * P:(mt + 1) * P],
                            out_t[:, kt, nk * 512:(nk + 1) * 512],
                            start=(kt == 0),
                            stop=(kt == KT3 - 1),
                        )
                    nc.vector.tensor_copy(y_sb[:, nk * 512:(nk + 1) * 512], ps_y[:])
                nc.sync.dma_start(y_d[mt * P:(mt + 1) * P, :], y_sb[:])


def _host_prep(x, w_qkv, q_gamma, q_beta, k_gamma, k_beta, w_proj):
    """Per-core input maps."""
    ident = np.eye(P, dtype=np.float32)
    sh64 = np.zeros((P, P), dtype=np.float32)
    sh64[(np.arange(P) + 64) % P, np.arange(P)] = 1.0
    bd6 = np.zeros((CL, HL), dtype=np.float32)
    for h in range(HL):
        bd6[h * D:(h + 1) * D, h] = 1.0
    bc6 = np.ascontiguousarray(bd6.T)
    in_maps = []
    for c in range(8):
        b = c // 2
        half = c % 2
        heads = range(HL * half, HL * half + HL)
        wq = np.concatenate([w_qkv[:, h * D:(h + 1) * D] for h in heads], axis=1)
        wk = np.concatenate(
            [w_qkv[:, C + h * D:C + (h + 1) * D] for h in heads], axis=1
        )
        wv = np.concatenate(
            [w_qkv[:, 2 * C + h * D:2 * C + (h + 1) * D] for h in heads], axis=1
        )
        wqk = np.ascontiguousarray(np.concatenate([wq, wk], axis=1))
        wp = np.ascontiguousarray(w_proj[CL * half:CL * half + CL, :])
        gb = np.stack(
            [
                np.tile(q_gamma, HL),
                np.tile(q_beta, HL) * SCALE,
                np.tile(k_gamma, HL),
                np.tile(k_beta, HL),
            ],
            axis=1,
        ).astype(np.float32)
        in_maps.append(
            {
                "x": np.ascontiguousarray(x[b]),
                "wqk": wqk,
                "wv": np.ascontiguousarray(wv),
                "wp": wp,
                "ident": ident,
                "sh64": sh64,
                "bd6": bd6,
                "bc6": bc6,
                "gb": gb,
            }
        )
    return in_maps


def kernel(x, w_qkv, q_gamma, q_beta, k_gamma, k_beta, w_proj, b_proj):
    x = np.asarray(x, dtype=np.float32)
    w_qkv = np.asarray(w_qkv, dtype=np.float32)
    q_gamma = np.asarray(q_gamma, dtype=np.float32)
    q_beta = np.asarray(q_beta, dtype=np.float32)
    k_gamma = np.asarray(k_gamma, dtype=np.float32)
    k_beta = np.asarray(k_beta, dtype=np.float32)
    w_proj = np.asarray(w_proj, dtype=np.float32)
    b_proj = np.asarray(b_proj, dtype=np.float32)

    trivial_beta = bool(np.all(q_beta == 0.0) and np.all(k_beta == 0.0))
    nc = _build(trivial_beta)
    in_maps = _host_prep(x, w_qkv, q_gamma, q_beta, k_gamma, k_beta, w_proj)
    res = run_bass_kernel_spmd(nc, in_maps, core_ids=list(range(8)))

    y = np.empty((B, N, C), dtype=np.float32)
    for b in range(B):
        yt = res.results[2 * b]["y"] + res.results[2 * b + 1]["y"]
        y[b] = yt.T + b_proj[None, :]
    return y


if __name__ == "__main__":
    rng = np.random.default_rng(0)
    out = kernel(
        rng.standard_normal((B, N, C), dtype=np.float32),
        (rng.standard_normal((C, 3 * C)) * C ** -0.5).astype(np.float32),
        np.ones(D, np.float32),
        np.zeros(D, np.float32),
        np.ones(D, np.float32),
        np.zeros(D, np.float32),
        (rng.standard_normal((C, C)) * C ** -0.5).astype(np.float32),
        np.zeros(C, np.float32),
    )
    print("ok", out.shape, float(np.abs(out).mean()))


# revision 27
# speedup vs baseline: 18.0370x; 1.3837x over previous
"""Trainium2 Bass kernel for nn_Attention_45835890982922.

Dense multi-head attention block:
    qkv = x @ w_qkv ; q,k layernormed per head (eps=1e-5), q scaled by D^-0.5
    out = softmax(q k^T) v ; y = concat_heads(out) @ w_proj + b_proj

Sharding over 8 NeuronCores: hybrid batch x tensor-parallel.
Core c handles batch b = c//2 and heads [6*(c%2), 6*(c%2)+6).
Each core computes a partial y^T (its 6 heads through the matching
w_proj rows); the host sums the two partials per batch and adds b_proj.

On-chip layout is feature-major (transposed): x^T, q^T, k^T [D, tokens],
so every matmul contraction lives on the partition axis with no per-head
transposes.  Softmax runs without max-subtraction (|S| <= ~8 after LN),
with the normalization sum obtained from an extra all-ones column
appended to v; the division is folded into the PSUM->SBUF epilogue of
the attention-output matmul.

dtypes: float32r (TensorE reduced fp32, ~1.5e-4) for qkv/S/stats/proj
matmuls, bf16 for exp(S) probabilities and v, fp32 accumulation in PSUM.
"""

from contextlib import ExitStack

import numpy as np

import concourse.bacc as bacc
import concourse.tile as tile
import concourse.mybir as mybir
from concourse.bass_utils import run_bass_kernel_spmd

F32 = mybir.dt.float32
F32R = mybir.dt.float32r
BF16 = mybir.dt.bfloat16
OP = mybir.AluOpType
AF = mybir.ActivationFunctionType

B, N, C, H, D = 4, 2048, 768, 12, 64
HL = H // 2              # 6 heads per core
CL = HL * D              # 384 local feature rows
P = 128
NKT = N // P             # 16 key tiles
QC = 1024                # query chunk for attention
NQC = N // QC
CT = C // P              # 6 contraction tiles over C
FT_QK = 2 * CL // P      # 6 feature tiles for q|k
KT3 = CL // P            # 3 contraction tiles over CL
LN_EPS = 1e-5
SCALE = float(D) ** -0.5

# ablation knobs (timing experiments only; wrong output when not default)
ABL_HEADS = HL
ABL_QKV = True
ABL_ATTN = True
ABL_PROJ = True
ABL_SHIFT = False
ABL_EXPFN = True
ABL_LN = True
PSS_BUFS = 2
PSO_BUFS = 2
HPARTS = 4
ABL_EPI = True


def _build(trivial_beta: bool, repeat: int = 1):
    nc = bacc.Bacc("TRN2", target_bir_lowering=False, debug=False, num_devices=8)

    x_d = nc.dram_tensor("x", [N, C], F32, kind="ExternalInput").ap()
    wqk_d = nc.dram_tensor("wqk", [C, 2 * CL], F32R, kind="ExternalInput").ap()
    wv_d = nc.dram_tensor("wv", [C, CL], F32R, kind="ExternalInput").ap()
    wp_d = nc.dram_tensor("wp", [CL, C], F32R, kind="ExternalInput").ap()
    ident_d = nc.dram_tensor("ident", [P, P], F32, kind="ExternalInput").ap()
    bd6_d = nc.dram_tensor("bd6", [CL, HL], F32R, kind="ExternalInput").ap()
    bc6_d = nc.dram_tensor("bc6", [HL, CL], F32R, kind="ExternalInput").ap()
    gb_d = nc.dram_tensor("gb", [CL, 4], F32, kind="ExternalInput").ap()
    sh_d = nc.dram_tensor("sh64", [P, P], F32R, kind="ExternalInput").ap()
    y_d = nc.dram_tensor("y", [C, N], F32, kind="ExternalOutput").ap()

    with tile.TileContext(nc) as tc, ExitStack() as top:
        top.enter_context(
            nc.allow_low_precision(reason="f32r/bf16 staging is intentional")
        )
        const = top.enter_context(tc.tile_pool(name="const", bufs=1))

        ident = const.tile([P, P], F32)
        nc.sync.dma_start(ident[:], ident_d)
        bd6 = const.tile([P, KT3, HL], F32R)
        nc.sync.dma_start(bd6[:], bd6_d.rearrange("(t p) h -> p t h", p=P))
        bc6 = const.tile([HL, CL], F32R)
        nc.sync.dma_start(bc6[:], bc6_d)
        gb = const.tile([P, KT3, 4], F32)
        nc.sync.dma_start(gb[:], gb_d.rearrange("(t p) c -> p t c", p=P))
        sh64 = const.tile([P, P], F32R)
        nc.sync.dma_start(sh64[:], sh_d)

        for rep in range(repeat):
            _emit_iteration(
                nc, tc, rep, trivial_beta,
                x_d, wqk_d, wv_d, wp_d, y_d, ident, bd6, bc6, gb, sh64,
            )

    nc.compile()
    return nc


def _emit_iteration(nc, tc, rep, trivial_beta,
                    x_d, wqk_d, wv_d, wp_d, y_d, ident, bd6, bc6, gb, sh64):
    with ExitStack() as top:
        vp = top.enter_context(tc.tile_pool(name=f"vpool{rep}", bufs=1))
        # v token-major bf16 with per-head all-ones column: [p, ttile, h*65+e]
        v_sb = vp.tile([P, NKT, HL * 65], BF16)
        v_view = v_sb[:].rearrange("p t (h e) -> p t h e", h=HL)
        nc.gpsimd.memset(v_view[:, :, :, 64:65], 1.0)

        # q^T | k^T feature-major accumulator: [p, ft, tokens]; ft 0-2 q, 3-5 k.
        # LayerNorm is applied in-place, so this same tile later holds qhat/khat.
        qkp = top.enter_context(tc.tile_pool(name=f"qkraw{rep}", bufs=1))
        qk_fts = [
            qkp.tile([P, N], F32R, name=f"qk_ft{ft}_{rep}") for ft in range(FT_QK)
        ]

        class _FtView:
            """hat[p_slice, ft, col_slice] -> per-ft tile AP."""
            def __init__(self, tiles):
                self.tiles = tiles
            def __getitem__(self, idx):
                p, ft, col = idx
                return self.tiles[ft][p, col]

        qk_raw = _FtView(qk_fts)
        # LN smalls live through phases B-C only
        sAC = top.enter_context(ExitStack())
        smp = sAC.enter_context(tc.tile_pool(name=f"smalls{rep}", bufs=1))

        # ================ phase A: x^T, qkv, v ================
        with ExitStack() as sA:
            pA = sA.enter_context(tc.tile_pool(name=f"phA{rep}", bufs=1))
            pAx = sA.enter_context(tc.tile_pool(name=f"phAx{rep}", bufs=2))
            psA = sA.enter_context(tc.tile_pool(name=f"psA{rep}", bufs=2, space="PSUM"))

            wqk_r = pA.tile([P, CT, 2 * CL], F32R)
            nc.sync.dma_start(wqk_r[:], wqk_d.rearrange("(t p) f -> p t f", p=P))
            wv_r = pA.tile([P, CT, CL], F32R)
            nc.sync.dma_start(wv_r[:], wv_d.rearrange("(t p) f -> p t f", p=P))

            x_t = pA.tile([P, CT, N], F32R)           # [c%128, ctile, token]
            for ct in range(CT):
                xs = pAx.tile([P, NKT, P], F32, tag="xslice")
                nc.sync.dma_start(
                    xs[:], x_d.rearrange("(t p) c -> p t c", p=P)[:, :, ct * P:(ct + 1) * P]
                )
                for tt in range(NKT):
                    pst = psA.tile([P, P], F32, tag="ps_tr")
                    nc.tensor.transpose(pst[:], xs[:, tt, :], ident[:])
                    nc.vector.tensor_copy(x_t[:, ct, tt * P:(tt + 1) * P], pst[:])

            for ft in range(FT_QK if ABL_QKV else 0):
                for nk in range(N // 512):
                    ps = psA.tile([P, 512], F32, tag="ps_qkv")
                    for kt in range(CT):
                        nc.tensor.matmul(
                            ps[:],
                            wqk_r[:, kt, ft * P:(ft + 1) * P],
                            x_t[:, kt, nk * 512:(nk + 1) * 512],
                            start=(kt == 0),
                            stop=(kt == CT - 1),
                        )
                    nc.vector.tensor_copy(qk_raw[:, ft, nk * 512:(nk + 1) * 512], ps[:])

            for tt in range(NKT):
                psv = psA.tile([P, CL], F32, tag="ps_v")
                for kt in range(CT):
                    nc.tensor.matmul(
                        psv[:],
                        x_t[:, kt, tt * P:(tt + 1) * P],
                        wv_r[:, kt, :],
                        start=(kt == 0),
                        stop=(kt == CT - 1),
                    )
                nc.vector.tensor_copy(
                    v_view[:, tt, :, 0:64],
                    psv[:].rearrange("p (h d) -> p h d", h=HL),
                )

        # ================ phase B: LN stats ================
        # per-token sums over D via block-diagonal ones matmuls -> [6, tokens]
        with ExitStack() as sB:
            pB = sB.enter_context(tc.tile_pool(name=f"phB{rep}", bufs=2))
            psB = sB.enter_context(tc.tile_pool(name=f"psB{rep}", bufs=2, space="PSUM"))

            sm_mu = [smp.tile([HL, N], F32R, tag=f"mu{s}", name=f"sm_mu{s}_{rep}") for s in range(2)]
            sm_rst = [smp.tile([HL, N], F32R, tag=f"rst{s}", name=f"sm_rst{s}_{rep}") for s in range(2)]

            for s in range(2 if ABL_LN else 0):
                for nk in range(N // 512):
                    psm = psB.tile([HL, 512], F32, tag="ps_stat")
                    for kt in range(KT3):
                        nc.tensor.matmul(
                            psm[:],
                            bd6[:, kt, :],
                            qk_raw[:, 3 * s + kt, nk * 512:(nk + 1) * 512],
                            start=(kt == 0),
                            stop=(kt == KT3 - 1),
                        )
                    nc.vector.tensor_scalar_mul(
                        sm_mu[s][:, nk * 512:(nk + 1) * 512], psm[:], 1.0 / D
                    )
                for nk in range(N // 512):
                    psm = psB.tile([HL, 512], F32, tag="ps_stat")
                    for kt in range(KT3):
                        sq = pB.tile([P, 512], F32R, tag="sq")
                        nc.scalar.square(
                            sq[:], qk_raw[:, 3 * s + kt, nk * 512:(nk + 1) * 512]
                        )
                        nc.tensor.matmul(
                            psm[:],
                            bd6[:, kt, :],
                            sq[:],
                            start=(kt == 0),
                            stop=(kt == KT3 - 1),
                        )
                    nc.vector.tensor_scalar_mul(
                        sm_rst[s][:, nk * 512:(nk + 1) * 512], psm[:], 1.0 / D
                    )
                # var = E[x^2] - mu^2 ; rstd = 1/sqrt(var+eps)
                tmp = smp.tile([HL, N], F32, tag=f"tmp{s}", name=f"tmp{s}_{rep}")
                nc.vector.tensor_tensor(tmp[:], sm_mu[s][:], sm_mu[s][:], OP.mult)
                nc.vector.scalar_tensor_tensor(
                    tmp[:], sm_rst[s][:], LN_EPS, tmp[:],
                    op0=OP.add, op1=OP.subtract,
                )
                nc.scalar.activation(tmp[:], tmp[:], AF.Sqrt)
                nc.vector.reciprocal(sm_rst[s][:], tmp[:])
                if s == 0:
                    nc.vector.tensor_scalar_mul(
                        sm_rst[0][:], sm_rst[0][:], SCALE
                    )

        # ================ phase C: LN apply (in-place into qk_raw) ============
        # qhat = ((raw - mu_bcast) * gamma) * rstd_bcast [+ beta]
        hat = qk_raw
        with ExitStack() as sC:
            pC = sC.enter_context(tc.tile_pool(name=f"phC{rep}", bufs=2))
            psC = sC.enter_context(tc.tile_pool(name=f"psC{rep}", bufs=2, space="PSUM"))
            for ft in ([0, 3, 1, 4, 2, 5][:FT_QK] if ABL_LN else []):
                s = ft // 3
                blk = ft % 3
                for nk in range(N // 512):
                    sl = slice(nk * 512, (nk + 1) * 512)
                    bmu = psC.tile([P, 512], F32, tag="bmu")
                    nc.tensor.matmul(
                        bmu[:], bc6[:, blk * P:(blk + 1) * P], sm_mu[s][:, sl],
                        start=True, stop=True,
                    )
                    brs = psC.tile([P, 512], F32, tag="brs")
                    nc.tensor.matmul(
                        brs[:], bc6[:, blk * P:(blk + 1) * P], sm_rst[s][:, sl],
                        start=True, stop=True,
                    )
                    tdiff = pC.tile([P, 512], F32, tag="tdiff")
                    nc.vector.tensor_tensor(
                        tdiff[:], qk_raw[:, ft, sl], bmu[:], OP.subtract
                    )
                    nc.vector.scalar_tensor_tensor(
                        hat[:, ft, sl],
                        tdiff[:],
                        gb[:, blk, 2 * s:2 * s + 1],
                        brs[:],
                        op0=OP.mult,
                        op1=OP.mult,
                    )
                    if not trivial_beta:
                        nc.vector.tensor_scalar_add(
                            hat[:, ft, sl], hat[:, ft, sl],
                            gb[:, blk, 2 * s + 1:2 * s + 2],
                        )

        sAC.close()  # free LN smalls before attention

        # ================ phase C2: rotated copy of qhat|khat ================
        # hat_sh[p] = hat[(p+64)%128]: kt-odd S matmuls read it so adjacent
        # S matmuls hit different PE row groups and overlap.
        shp = top.enter_context(tc.tile_pool(name=f"shp{rep}", bufs=1))
        hat_sh = shp.tile([P, FT_QK, N], F32R, name=f"hat_sh{rep}") if ABL_SHIFT else hat
        with ExitStack() as sC2:
            psC2 = sC2.enter_context(tc.tile_pool(name=f"psC2{rep}", bufs=2, space="PSUM"))
            for t in range(FT_QK if ABL_SHIFT else 0):
                for nk in range(N // 512):
                    pssh = psC2.tile([P, 512], F32, tag="ps_sh")
                    nc.tensor.matmul(
                        pssh[:], sh64[:],
                        hat[:, t, nk * 512:(nk + 1) * 512],
                        start=True, stop=True,
                    )
                    nc.vector.tensor_copy(
                        hat_sh[:, t, nk * 512:(nk + 1) * 512], pssh[:]
                    )

        # ================ phase D: attention ================
        outp = top.enter_context(tc.tile_pool(name=f"outT{rep}", bufs=1))
        out_fts = [
            outp.tile([P, N], F32R, name=f"out_ft{t}_{rep}") for t in range(KT3)
        ]
        out_t = _FtView(out_fts)                      # out^T feature-major
        with ExitStack() as sD:
            expp = sD.enter_context(tc.tile_pool(name=f"expp{rep}", bufs=2 * HPARTS))
            pD = sD.enter_context(tc.tile_pool(name=f"phD{rep}", bufs=2))
            psS = sD.enter_context(tc.tile_pool(name=f"psS{rep}", bufs=PSS_BUFS, space="PSUM"))
            psO = sD.enter_context(tc.tile_pool(name=f"psO{rep}", bufs=PSO_BUFS, space="PSUM"))
            HK = NKT // HPARTS

            def emit_s_exp(h, qc):
                ht = h // 2
                hr = 64 * (h % 2)
                exp_halves = []
                for half in range(HPARTS):
                    exp_h = expp.tile(
                        [P, HK, QC], BF16, tag="exp", name=f"exp_{rep}_{h}_{qc}_{half}"
                    )
                    exp_halves.append(exp_h)
                    for kt in range(half * HK, (half + 1) * HK):
                        ps_st = psS.tile([P, QC], F32, tag="ps_s")
                        if kt % 2 == 0 or not ABL_SHIFT:
                            lhs = hat[hr:hr + 64, 3 + ht, kt * P:(kt + 1) * P]
                            rhs = hat[hr:hr + 64, ht, qc * QC:(qc + 1) * QC]
                        else:
                            lhs = hat_sh[64 - hr:128 - hr, 3 + ht,
                                         kt * P:(kt + 1) * P]
                            rhs = hat_sh[64 - hr:128 - hr, ht,
                                         qc * QC:(qc + 1) * QC]
                        for nk in range(QC // 512):
                            nc.tensor.matmul(
                                ps_st[:, nk * 512:(nk + 1) * 512],
                                lhs,
                                rhs[:, nk * 512:(nk + 1) * 512],
                                start=True,
                                stop=True,
                            )
                        nc.scalar.activation(
                            exp_h[:, kt - half * HK, :], ps_st[:],
                            AF.Exp if ABL_EXPFN else AF.Copy,
                        )
                return exp_halves

            def emit_pv(h, qc, exp_halves):
                ht = h // 2
                hr = 64 * (h % 2)
                ps_o = psO.tile([65, QC], F32, tag="ps_o")
                for kt in range(NKT):
                    for nk in range(QC // 512):
                        nc.tensor.matmul(
                            ps_o[:, nk * 512:(nk + 1) * 512],
                            v_view[:, kt, h, :],
                            exp_halves[kt // HK][:, kt % HK,
                                                 nk * 512:(nk + 1) * 512],
                            start=(kt == 0),
                            stop=(kt == NKT - 1),
                        )
                if ABL_EPI:
                    rc = pD.tile([1, QC], F32, tag="recip")
                    nc.vector.reciprocal(rc[:], ps_o[64:65, :])
                    rcb = pD.tile([64, QC], F32, tag="recipb")
                    nc.gpsimd.partition_broadcast(rcb[:], rc[:])
                    nc.vector.tensor_tensor(
                        out_t[hr:hr + 64, ht, qc * QC:(qc + 1) * QC],
                        ps_o[0:64, :],
                        rcb[:],
                        OP.mult,
                    )
                else:
                    nc.vector.tensor_copy(
                        out_t[hr:hr + 64, ht, qc * QC:(qc + 1) * QC],
                        ps_o[0:64, :],
                    )

            # software pipeline: next chunk's S/exp is emitted before this
            # chunk's PV so the PE feeds ACT continuously.
            pending = None
            for h in range(ABL_HEADS if ABL_ATTN else 0):
                for qc in range(NQC):
                    eh = emit_s_exp(h, qc)
                    if pending is not None:
                        emit_pv(*pending)
                    pending = (h, qc, eh)
            if pending is not None:
                emit_pv(*pending)

        # ================ phase E: output projection ================
        with ExitStack() as sE:
            pE = sE.enter_context(tc.tile_pool(name=f"phE{rep}", bufs=2))
            wpp = sE.enter_context(tc.tile_pool(name=f"wpp{rep}", bufs=1))
            psE = sE.enter_context(tc.tile_pool(name=f"psE{rep}", bufs=2, space="PSUM"))
            wp_r = wpp.tile([P, KT3, C], F32R)
            nc.sync.dma_start(wp_r[:], wp_d.rearrange("(t p) f -> p t f", p=P))
            for mt in range(C // P if ABL_PROJ else 0):
                y_sb = pE.tile([P, N], F32, tag="y")
                for nk in range(N // 512):
                    ps_y = psE.tile([P, 512], F32, tag="ps_y")
                    for kt in range(KT3):
                        nc.tensor.matmul(
                            ps_y[:],
                            wp_r[:, kt, mt * P:(mt + 1) * P],
                            out_t[:, kt, nk * 512:(nk + 1) * 512],
                            start=(kt == 0),
                            stop=(kt == KT3 - 1),
                        )
                    nc.vector.tensor_copy(y_sb[:, nk * 512:(nk + 1) * 512], ps_y[:])
                nc.sync.dma_start(y_d[mt * P:(mt + 1) * P, :], y_sb[:])


def _host_prep(x, w_qkv, q_gamma, q_beta, k_gamma, k_beta, w_proj):
    """Per-core input maps."""
    ident = np.eye(P, dtype=np.float32)
    sh64 = np.zeros((P, P), dtype=np.float32)
    sh64[(np.arange(P) + 64) % P, np.arange(P)] = 1.0
    bd6 = np.zeros((CL, HL), dtype=np.float32)
    for h in range(HL):
        bd6[h * D:(h + 1) * D, h] = 1.0
    bc6 = np.ascontiguousarray(bd6.T)
    in_maps = []
    for c in range(8):
        b = c // 2
        half = c % 2
        heads = range(HL * half, HL * half + HL)
        wq = np.concatenate([w_qkv[:, h * D:(h + 1) * D] for h in heads], axis=1)
        wk = np.concatenate(
            [w_qkv[:, C + h * D:C + (h + 1) * D] for h in heads], axis=1
        )
        wv = np.concatenate(
            [w_qkv[:, 2 * C + h * D:2 * C + (h + 1) * D] for h in heads], axis=1
        )
        wqk = np.ascontiguousarray(np.concatenate([wq, wk], axis=1))
        wp = np.ascontiguousarray(w_proj[CL * half:CL * half + CL, :])
        gb = np.stack(
            [
                np.tile(q_gamma, HL),
                np.tile(q_beta, HL) * SCALE,
                np.tile(k_gamma, HL),
                np.tile(k_beta, HL),
            ],
            axis=1,
        ).astype(np.float32)
        in_maps.append(
            {
                "x": np.ascontiguousarray(x[b]),
                "wqk": wqk,
                "wv": np.ascontiguousarray(wv),
                "wp": wp,
                "ident": ident,
                "sh64": sh64,
                "bd6": bd6,
                "bc6": bc6,
                "gb": gb,
            }
        )
    return in_maps


def kernel(x, w_qkv, q_gamma, q_beta, k_gamma, k_beta, w_proj, b_proj):
    x = np.asarray(x, dtype=np.float32)
    w_qkv = np.asarray(w_qkv, dtype=np.float32)
    q_gamma = np.asarray(q_gamma, dtype=np.float32)
    q_beta = np.asarray(q_beta, dtype=np.float32)
    k_gamma = np.asarray(k_gamma, dtype=np.float32)
    k_beta = np.asarray(k_beta, dtype=np.float32)
    w_proj = np.asarray(w_proj, dtype=np.float32)
    b_proj = np.asarray(b_proj, dtype=np.float32)

    trivial_beta = bool(np.all(q_beta == 0.0) and np.all(k_beta == 0.0))
    nc = _build(trivial_beta)
    in_maps = _host_prep(x, w_qkv, q_gamma, q_beta, k_gamma, k_beta, w_proj)
    res = run_bass_kernel_spmd(nc, in_maps, core_ids=list(range(8)))

    y = np.empty((B, N, C), dtype=np.float32)
    for b in range(B):
        yt = res.results[2 * b]["y"] + res.results[2 * b + 1]["y"]
        y[b] = yt.T + b_proj[None, :]
    return y


if __name__ == "__main__":
    rng = np.random.default_rng(0)
    out = kernel(
        rng.standard_normal((B, N, C), dtype=np.float32),
        (rng.standard_normal((C, 3 * C)) * C ** -0.5).astype(np.float32),
        np.ones(D, np.float32),
        np.zeros(D, np.float32),
        np.ones(D, np.float32),
        np.zeros(D, np.float32),
        (rng.standard_normal((C, C)) * C ** -0.5).astype(np.float32),
        np.zeros(C, np.float32),
    )
    print("ok", out.shape, float(np.abs(out).mean()))


# revision 33
# speedup vs baseline: 18.9275x; 1.0494x over previous
"""Trainium2 Bass kernel for nn_Attention_45835890982922.

Dense multi-head attention block:
    qkv = x @ w_qkv ; q,k layernormed per head (eps=1e-5), q scaled by D^-0.5
    out = softmax(q k^T) v ; y = concat_heads(out) @ w_proj + b_proj

Sharding over 8 NeuronCores: hybrid batch x tensor-parallel.
Core c handles batch b = c//2 and heads [6*(c%2), 6*(c%2)+6).
Each core computes a partial y^T (its 6 heads through the matching
w_proj rows); the host sums the two partials per batch and adds b_proj.

On-chip layout is feature-major (transposed): x^T, q^T, k^T [D, tokens],
so every matmul contraction lives on the partition axis with no per-head
transposes.  Softmax runs without max-subtraction (|S| <= ~8 after LN),
with the normalization sum obtained from an extra all-ones column
appended to v; the division is folded into the PSUM->SBUF epilogue of
the attention-output matmul.

dtypes: float32r (TensorE reduced fp32, ~1.5e-4) for qkv/S/stats/proj
matmuls, bf16 for exp(S) probabilities and v, fp32 accumulation in PSUM.
"""

from contextlib import ExitStack

import numpy as np

import concourse.bacc as bacc
import concourse.tile as tile
import concourse.mybir as mybir
from concourse.bass_utils import run_bass_kernel_spmd

F32 = mybir.dt.float32
F32R = mybir.dt.float32r
BF16 = mybir.dt.bfloat16
OP = mybir.AluOpType
AF = mybir.ActivationFunctionType

B, N, C, H, D = 4, 2048, 768, 12, 64
HL = H // 2              # 6 heads per core
CL = HL * D              # 384 local feature rows
P = 128
NKT = N // P             # 16 key tiles
QC = 1024                # query chunk for attention
NQC = N // QC
CT = C // P              # 6 contraction tiles over C
FT_QK = 2 * CL // P      # 6 feature tiles for q|k
KT3 = CL // P            # 3 contraction tiles over CL
LN_EPS = 1e-5
SCALE = float(D) ** -0.5

# ablation knobs (timing experiments only; wrong output when not default)
ABL_HEADS = HL
ABL_QKV = True
ABL_ATTN = True
ABL_PROJ = True
ABL_SHIFT = False
ABL_EXPFN = True
ABL_LN = True
PSS_BUFS = 2
PSO_BUFS = 2
HPARTS = 4
ABL_EPI = True


def _build(trivial_beta: bool, repeat: int = 1):
    nc = bacc.Bacc("TRN2", target_bir_lowering=False, debug=False, num_devices=8)

    x_d = nc.dram_tensor("x", [N, C], F32, kind="ExternalInput").ap()
    wqk_d = nc.dram_tensor("wqk", [C, 2 * CL], F32R, kind="ExternalInput").ap()
    wv_d = nc.dram_tensor("wv", [C, CL], F32R, kind="ExternalInput").ap()
    wp_d = nc.dram_tensor("wp", [CL, C], F32R, kind="ExternalInput").ap()
    ident_d = nc.dram_tensor("ident", [P, P], F32, kind="ExternalInput").ap()
    bd6_d = nc.dram_tensor("bd6", [CL, HL], F32R, kind="ExternalInput").ap()
    bc6_d = nc.dram_tensor("bc6", [HL, CL], F32R, kind="ExternalInput").ap()
    gb_d = nc.dram_tensor("gb", [CL, 4], F32, kind="ExternalInput").ap()
    sh_d = nc.dram_tensor("sh64", [P, P], F32R, kind="ExternalInput").ap()
    y_d = nc.dram_tensor("y", [C, N], F32, kind="ExternalOutput").ap()

    with tile.TileContext(nc) as tc, ExitStack() as top:
        top.enter_context(
            nc.allow_low_precision(reason="f32r/bf16 staging is intentional")
        )
        const = top.enter_context(tc.tile_pool(name="const", bufs=1))

        ident = const.tile([P, P], F32)
        nc.sync.dma_start(ident[:], ident_d)
        bd6 = const.tile([P, KT3, HL], F32R)
        nc.sync.dma_start(bd6[:], bd6_d.rearrange("(t p) h -> p t h", p=P))
        bc6 = const.tile([HL, CL], F32R)
        nc.sync.dma_start(bc6[:], bc6_d)
        gb = const.tile([P, KT3, 4], F32)
        nc.sync.dma_start(gb[:], gb_d.rearrange("(t p) c -> p t c", p=P))
        sh64 = const.tile([P, P], F32R)
        nc.sync.dma_start(sh64[:], sh_d)

        for rep in range(repeat):
            _emit_iteration(
                nc, tc, rep, trivial_beta,
                x_d, wqk_d, wv_d, wp_d, y_d, ident, bd6, bc6, gb, sh64,
            )

    nc.compile()
    return nc


def _emit_iteration(nc, tc, rep, trivial_beta,
                    x_d, wqk_d, wv_d, wp_d, y_d, ident, bd6, bc6, gb, sh64):
    with ExitStack() as top:
        vp = top.enter_context(tc.tile_pool(name=f"vpool{rep}", bufs=1))
        # v token-major bf16 with per-head all-ones column: [p, ttile, h*65+e]
        v_sb = vp.tile([P, NKT, HL * 65], BF16)
        v_view = v_sb[:].rearrange("p t (h e) -> p t h e", h=HL)
        nc.gpsimd.memset(v_view[:, :, :, 64:65], 1.0)

        # q^T | k^T feature-major accumulator: [p, ft, tokens]; ft 0-2 q, 3-5 k.
        # LayerNorm is applied in-place, so this same tile later holds qhat/khat.
        qkp = top.enter_context(tc.tile_pool(name=f"qkraw{rep}", bufs=1))
        qk_fts = [
            qkp.tile([P, N], F32R, name=f"qk_ft{ft}_{rep}") for ft in range(FT_QK)
        ]

        class _FtView:
            """hat[p_slice, ft, col_slice] -> per-ft tile AP."""
            def __init__(self, tiles):
                self.tiles = tiles
            def __getitem__(self, idx):
                p, ft, col = idx
                return self.tiles[ft][p, col]

        qk_raw = _FtView(qk_fts)
        # LN smalls live through phases B-C only
        sAC = top.enter_context(ExitStack())
        smp = sAC.enter_context(tc.tile_pool(name=f"smalls{rep}", bufs=1))

        # ================ phase A: x^T, qkv, v ================
        with ExitStack() as sA:
            pA = sA.enter_context(tc.tile_pool(name=f"phA{rep}", bufs=1))
            pAx = sA.enter_context(tc.tile_pool(name=f"phAx{rep}", bufs=2))
            psA = sA.enter_context(tc.tile_pool(name=f"psA{rep}", bufs=2, space="PSUM"))

            wqk_r = pA.tile([P, CT, 2 * CL], F32R)
            nc.sync.dma_start(wqk_r[:], wqk_d.rearrange("(t p) f -> p t f", p=P))
            wv_r = pA.tile([P, CT, CL], F32R)
            nc.sync.dma_start(wv_r[:], wv_d.rearrange("(t p) f -> p t f", p=P))

            x_t = pA.tile([P, CT, N], F32R)           # [c%128, ctile, token]
            for ct in range(CT):
                xs = pAx.tile([P, NKT, P], F32, tag="xslice")
                nc.sync.dma_start(
                    xs[:], x_d.rearrange("(t p) c -> p t c", p=P)[:, :, ct * P:(ct + 1) * P]
                )
                for tt in range(NKT):
                    pst = psA.tile([P, P], F32, tag="ps_tr")
                    nc.tensor.transpose(pst[:], xs[:, tt, :], ident[:])
                    nc.vector.tensor_copy(x_t[:, ct, tt * P:(tt + 1) * P], pst[:])

            for ft in range(FT_QK if ABL_QKV else 0):
                for nk in range(N // 512):
                    ps = psA.tile([P, 512], F32, tag="ps_qkv")
                    for kt in range(CT):
                        nc.tensor.matmul(
                            ps[:],
                            wqk_r[:, kt, ft * P:(ft + 1) * P],
                            x_t[:, kt, nk * 512:(nk + 1) * 512],
                            start=(kt == 0),
                            stop=(kt == CT - 1),
                        )
                    nc.vector.tensor_copy(qk_raw[:, ft, nk * 512:(nk + 1) * 512], ps[:])

            for tt in range(NKT):
                psv = psA.tile([P, CL], F32, tag="ps_v")
                for kt in range(CT):
                    nc.tensor.matmul(
                        psv[:],
                        x_t[:, kt, tt * P:(tt + 1) * P],
                        wv_r[:, kt, :],
                        start=(kt == 0),
                        stop=(kt == CT - 1),
                    )
                nc.vector.tensor_copy(
                    v_view[:, tt, :, 0:64],
                    psv[:].rearrange("p (h d) -> p h d", h=HL),
                )

        # ================ phase B: LN stats ================
        # per-token sums over D via block-diagonal ones matmuls -> [6, tokens]
        with ExitStack() as sB:
            pB = sB.enter_context(tc.tile_pool(name=f"phB{rep}", bufs=2))
            psB = sB.enter_context(tc.tile_pool(name=f"psB{rep}", bufs=2, space="PSUM"))

            sm_mu = [smp.tile([HL, N], F32R, tag=f"mu{s}", name=f"sm_mu{s}_{rep}") for s in range(2)]
            sm_rst = [smp.tile([HL, N], F32R, tag=f"rst{s}", name=f"sm_rst{s}_{rep}") for s in range(2)]

            for s in range(2 if ABL_LN else 0):
                for nk in range(N // 512):
                    psm = psB.tile([HL, 512], F32, tag="ps_stat")
                    for kt in range(KT3):
                        nc.tensor.matmul(
                            psm[:],
                            bd6[:, kt, :],
                            qk_raw[:, 3 * s + kt, nk * 512:(nk + 1) * 512],
                            start=(kt == 0),
                            stop=(kt == KT3 - 1),
                        )
                    nc.vector.tensor_scalar_mul(
                        sm_mu[s][:, nk * 512:(nk + 1) * 512], psm[:], 1.0 / D
                    )
                for nk in range(N // 512):
                    psm = psB.tile([HL, 512], F32, tag="ps_stat")
                    for kt in range(KT3):
                        sq = pB.tile([P, 512], F32R, tag="sq")
                        nc.scalar.square(
                            sq[:], qk_raw[:, 3 * s + kt, nk * 512:(nk + 1) * 512]
                        )
                        nc.tensor.matmul(
                            psm[:],
                            bd6[:, kt, :],
                            sq[:],
                            start=(kt == 0),
                            stop=(kt == KT3 - 1),
                        )
                    nc.vector.tensor_scalar_mul(
                        sm_rst[s][:, nk * 512:(nk + 1) * 512], psm[:], 1.0 / D
                    )
                # var = E[x^2] - mu^2 ; rstd = 1/sqrt(var+eps)
                tmp = smp.tile([HL, N], F32, tag=f"tmp{s}", name=f"tmp{s}_{rep}")
                nc.vector.tensor_tensor(tmp[:], sm_mu[s][:], sm_mu[s][:], OP.mult)
                nc.vector.scalar_tensor_tensor(
                    tmp[:], sm_rst[s][:], LN_EPS, tmp[:],
                    op0=OP.add, op1=OP.subtract,
                )
                nc.scalar.activation(tmp[:], tmp[:], AF.Sqrt)
                nc.vector.reciprocal(sm_rst[s][:], tmp[:])
                if s == 0:
                    nc.vector.tensor_scalar_mul(
                        sm_rst[0][:], sm_rst[0][:], SCALE
                    )

        # ================ phase C: LN apply (in-place into qk_raw) ============
        # qhat = ((raw - mu_bcast) * gamma) * rstd_bcast [+ beta]
        hat = qk_raw
        with ExitStack() as sC:
            pC = sC.enter_context(tc.tile_pool(name=f"phC{rep}", bufs=2))
            psC = sC.enter_context(tc.tile_pool(name=f"psC{rep}", bufs=2, space="PSUM"))
            for ft in ([0, 3, 1, 4, 2, 5][:FT_QK] if ABL_LN else []):
                s = ft // 3
                blk = ft % 3
                for nk in range(N // 512):
                    sl = slice(nk * 512, (nk + 1) * 512)
                    bmu = psC.tile([P, 512], F32, tag="bmu")
                    nc.tensor.matmul(
                        bmu[:], bc6[:, blk * P:(blk + 1) * P], sm_mu[s][:, sl],
                        start=True, stop=True,
                    )
                    brs = psC.tile([P, 512], F32, tag="brs")
                    nc.tensor.matmul(
                        brs[:], bc6[:, blk * P:(blk + 1) * P], sm_rst[s][:, sl],
                        start=True, stop=True,
                    )
                    tdiff = pC.tile([P, 512], F32, tag="tdiff")
                    nc.vector.tensor_tensor(
                        tdiff[:], qk_raw[:, ft, sl], bmu[:], OP.subtract
                    )
                    nc.vector.scalar_tensor_tensor(
                        hat[:, ft, sl],
                        tdiff[:],
                        gb[:, blk, 2 * s:2 * s + 1],
                        brs[:],
                        op0=OP.mult,
                        op1=OP.mult,
                    )
                    if not trivial_beta:
                        nc.vector.tensor_scalar_add(
                            hat[:, ft, sl], hat[:, ft, sl],
                            gb[:, blk, 2 * s + 1:2 * s + 2],
                        )

        sAC.close()  # free LN smalls before attention

        # ================ phase C2: rotated copy of qhat|khat ================
        # hat_sh[p] = hat[(p+64)%128]: kt-odd S matmuls read it so adjacent
        # S matmuls hit different PE row groups and overlap.
        shp = top.enter_context(tc.tile_pool(name=f"shp{rep}", bufs=1))
        hat_sh = shp.tile([P, FT_QK, N], F32R, name=f"hat_sh{rep}") if ABL_SHIFT else hat
        with ExitStack() as sC2:
            psC2 = sC2.enter_context(tc.tile_pool(name=f"psC2{rep}", bufs=2, space="PSUM"))
            for t in range(FT_QK if ABL_SHIFT else 0):
                for nk in range(N // 512):
                    pssh = psC2.tile([P, 512], F32, tag="ps_sh")
                    nc.tensor.matmul(
                        pssh[:], sh64[:],
                        hat[:, t, nk * 512:(nk + 1) * 512],
                        start=True, stop=True,
                    )
                    nc.vector.tensor_copy(
                        hat_sh[:, t, nk * 512:(nk + 1) * 512], pssh[:]
                    )

        # ================ phase D: attention ================
        outp = top.enter_context(tc.tile_pool(name=f"outT{rep}", bufs=1))
        out_fts = [
            outp.tile([P, N], F32R, name=f"out_ft{t}_{rep}") for t in range(KT3)
        ]
        out_t = _FtView(out_fts)                      # out^T feature-major
        with ExitStack() as sD:
            expp = sD.enter_context(tc.tile_pool(name=f"expp{rep}", bufs=2 * HPARTS))
            pD = sD.enter_context(tc.tile_pool(name=f"phD{rep}", bufs=2))
            psS = sD.enter_context(tc.tile_pool(name=f"psS{rep}", bufs=PSS_BUFS, space="PSUM"))
            psO = sD.enter_context(tc.tile_pool(name=f"psO{rep}", bufs=PSO_BUFS, space="PSUM"))
            HK = NKT // HPARTS

            def emit_s_exp(h, qc):
                ht = h // 2
                hr = 64 * (h % 2)
                exp_halves = []
                for half in range(HPARTS):
                    exp_h = expp.tile(
                        [P, HK, QC], BF16, tag="exp", name=f"exp_{rep}_{h}_{qc}_{half}"
                    )
                    exp_halves.append(exp_h)
                    for kt in range(half * HK, (half + 1) * HK):
                        ps_st = psS.tile([P, QC], F32, tag="ps_s")
                        if kt % 2 == 0 or not ABL_SHIFT:
                            lhs = hat[hr:hr + 64, 3 + ht, kt * P:(kt + 1) * P]
                            rhs = hat[hr:hr + 64, ht, qc * QC:(qc + 1) * QC]
                        else:
                            lhs = hat_sh[64 - hr:128 - hr, 3 + ht,
                                         kt * P:(kt + 1) * P]
                            rhs = hat_sh[64 - hr:128 - hr, ht,
                                         qc * QC:(qc + 1) * QC]
                        for nk in range(QC // 512):
                            nc.tensor.matmul(
                                ps_st[:, nk * 512:(nk + 1) * 512],
                                lhs,
                                rhs[:, nk * 512:(nk + 1) * 512],
                                start=True,
                                stop=True,
                            )
                        nc.scalar.activation(
                            exp_h[:, kt - half * HK, :], ps_st[:],
                            AF.Exp if ABL_EXPFN else AF.Copy,
                        )
                return exp_halves

            def emit_pv(h, qc, exp_halves):
                ht = h // 2
                hr = 64 * (h % 2)
                ps_o = psO.tile([65, QC], F32, tag="ps_o")
                for kt in range(NKT):
                    for nk in range(QC // 512):
                        nc.tensor.matmul(
                            ps_o[:, nk * 512:(nk + 1) * 512],
                            v_view[:, kt, h, :],
                            exp_halves[kt // HK][:, kt % HK,
                                                 nk * 512:(nk + 1) * 512],
                            start=(kt == 0),
                            stop=(kt == NKT - 1),
                        )
                if ABL_EPI:
                    rc = pD.tile([1, QC], F32, tag="recip")
                    nc.vector.reciprocal(rc[:], ps_o[64:65, :])
                    rcb = pD.tile([64, QC], F32, tag="recipb")
                    nc.gpsimd.partition_broadcast(rcb[:], rc[:])
                    nc.vector.tensor_tensor(
                        out_t[hr:hr + 64, ht, qc * QC:(qc + 1) * QC],
                        ps_o[0:64, :],
                        rcb[:],
                        OP.mult,
                    )
                else:
                    nc.vector.tensor_copy(
                        out_t[hr:hr + 64, ht, qc * QC:(qc + 1) * QC],
                        ps_o[0:64, :],
                    )

            # software pipeline: next chunk's S/exp is emitted before this
            # chunk's PV so the PE feeds ACT continuously.
            pending = None
            for h in range(ABL_HEADS if ABL_ATTN else 0):
                for qc in range(NQC):
                    eh = emit_s_exp(h, qc)
                    if pending is not None:
                        emit_pv(*pending)
                    pending = (h, qc, eh)
            if pending is not None:
                emit_pv(*pending)

        # ================ phase E: output projection ================
        with ExitStack() as sE:
            pE = sE.enter_context(tc.tile_pool(name=f"phE{rep}", bufs=2))
            wpp = sE.enter_context(tc.tile_pool(name=f"wpp{rep}", bufs=1))
            psE = sE.enter_context(tc.tile_pool(name=f"psE{rep}", bufs=2, space="PSUM"))
            wp_r = wpp.tile([P, KT3, C], F32R)
            nc.sync.dma_start(wp_r[:], wp_d.rearrange("(t p) f -> p t f", p=P))
            for mt in range(C // P if ABL_PROJ else 0):
                y_sb = pE.tile([P, N], F32, tag="y")
                for nk in range(N // 512):
                    ps_y = psE.tile([P, 512], F32, tag="ps_y")
                    for kt in range(KT3):
                        nc.tensor.matmul(
                            ps_y[:],
                            wp_r[:, kt, mt * P:(mt + 1) * P],
                            out_t[:, kt, nk * 512:(nk + 1) * 512],
                            start=(kt == 0),
                            stop=(kt == KT3 - 1),
                        )
                    nc.vector.tensor_copy(y_sb[:, nk * 512:(nk + 1) * 512], ps_y[:])
                nc.sync.dma_start(y_d[mt * P:(mt + 1) * P, :], y_sb[:])


def _host_prep(x, w_qkv, q_gamma, q_beta, k_gamma, k_beta, w_proj):
    """Per-core input maps."""
    ident = np.eye(P, dtype=np.float32)
    sh64 = np.zeros((P, P), dtype=np.float32)
    sh64[(np.arange(P) + 64) % P, np.arange(P)] = 1.0
    bd6 = np.zeros((CL, HL), dtype=np.float32)
    for h in range(HL):
        bd6[h * D:(h + 1) * D, h] = 1.0
    bc6 = np.ascontiguousarray(bd6.T)
    in_maps = []
    for c in range(8):
        b = c // 2
        half = c % 2
        heads = range(HL * half, HL * half + HL)
        wq = np.concatenate([w_qkv[:, h * D:(h + 1) * D] for h in heads], axis=1)
        wk = np.concatenate(
            [w_qkv[:, C + h * D:C + (h + 1) * D] for h in heads], axis=1
        )
        wv = np.concatenate(
            [w_qkv[:, 2 * C + h * D:2 * C + (h + 1) * D] for h in heads], axis=1
        )
        wqk = np.ascontiguousarray(np.concatenate([wq, wk], axis=1))
        wp = np.ascontiguousarray(w_proj[CL * half:CL * half + CL, :])
        gb = np.stack(
            [
                np.tile(q_gamma, HL),
                np.tile(q_beta, HL) * SCALE,
                np.tile(k_gamma, HL),
                np.tile(k_beta, HL),
            ],
            axis=1,
        ).astype(np.float32)
        in_maps.append(
            {
                "x": np.ascontiguousarray(x[b]),
                "wqk": wqk,
                "wv": np.ascontiguousarray(wv),
                "wp": wp,
                "ident": ident,
                "sh64": sh64,
                "bd6": bd6,
                "bc6": bc6,
                "gb": gb,
            }
        )
    return in_maps


def kernel(x, w_qkv, q_gamma, q_beta, k_gamma, k_beta, w_proj, b_proj):
    x = np.asarray(x, dtype=np.float32)
    w_qkv = np.asarray(w_qkv, dtype=np.float32)
    q_gamma = np.asarray(q_gamma, dtype=np.float32)
    q_beta = np.asarray(q_beta, dtype=np.float32)
    k_gamma = np.asarray(k_gamma, dtype=np.float32)
    k_beta = np.asarray(k_beta, dtype=np.float32)
    w_proj = np.asarray(w_proj, dtype=np.float32)
    b_proj = np.asarray(b_proj, dtype=np.float32)

    trivial_beta = bool(np.all(q_beta == 0.0) and np.all(k_beta == 0.0))
    nc = _build(trivial_beta)
    in_maps = _host_prep(x, w_qkv, q_gamma, q_beta, k_gamma, k_beta, w_proj)
    res = run_bass_kernel_spmd(nc, in_maps, core_ids=list(range(8)))

    y = np.empty((B, N, C), dtype=np.float32)
    for b in range(B):
        yt = res.results[2 * b]["y"] + res.results[2 * b + 1]["y"]
        y[b] = yt.T + b_proj[None, :]
    return y


if __name__ == "__main__":
    rng = np.random.default_rng(0)
    out = kernel(
        rng.standard_normal((B, N, C), dtype=np.float32),
        (rng.standard_normal((C, 3 * C)) * C ** -0.5).astype(np.float32),
        np.ones(D, np.float32),
        np.zeros(D, np.float32),
        np.ones(D, np.float32),
        np.zeros(D, np.float32),
        (rng.standard_normal((C, C)) * C ** -0.5).astype(np.float32),
        np.zeros(C, np.float32),
    )
    print("ok", out.shape, float(np.abs(out).mean()))
